# revision 5
# baseline (speedup 1.0000x reference)
"""AdaptiveTripletMarginLoss on 8 TRN2 NeuronCores — pure data-parallel.

Inputs: anchor/positive/negative [65536, 256] f32. Output: scalar mean loss.

Strategy:
  - Shard batch 8 ways (8192 samples/core).
  - Per core: for each 1024-sample big-tile, DMA a/p/n [128, 2048] f32,
    DVE computes w1 = a-p, w2 = a-n (bf16 out), then per 256-col slice:
    ACT Square+accum -> s11/s22 columns, DVE mult+reduce -> s12 columns.
    d_pn^2 = s11 + s22 - 2*s12 (identity: p-n = (a-n) - (a-p)).
  - Epilogue on [128, 64]: sqrt/exp/reciprocal -> per-sample loss, row-sum
    to [128,1], DMA out. Host sums 8x[128,1] and divides by B (adds the
    +2 constant from the two margin "1 +" terms).
"""

import sys

for _p in ("/opt/trn_rl_repo",):
    if _p not in sys.path:
        sys.path.insert(0, _p)

import numpy as np

import concourse.bass as bass  # noqa: F401
from concourse import bacc, bass_utils, mybir
import concourse.tile as tile

B, D = 65536, 256
NCORES = 8
BS = B // NCORES  # 8192 samples per core
P = 128  # SBUF partitions
SPT = 8  # samples per partition per big-tile
G = SPT * D  # 2048 free columns per big-tile
NT = BS // (P * SPT)  # 8 big-tiles per core
NCOLS = NT * SPT  # 64 accumulator columns
EPS = 1e-6

F32 = mybir.dt.float32
BF16 = mybir.dt.bfloat16
Alu = mybir.AluOpType
Act = mybir.ActivationFunctionType
AX = mybir.AxisListType

_CACHE = {}

# Which of the SPT slices per big-tile route their s11/s22 square+reduce to
# DVE (scalar_tensor_tensor) instead of ACT, to balance engine load.
DVE_SQUARE_SLICES = ()


def _build():
    nc = bacc.Bacc("TRN2", target_bir_lowering=False, debug=False, num_devices=NCORES)

    a_h = nc.dram_tensor("anchor", [BS, D], F32, kind="ExternalInput")
    p_h = nc.dram_tensor("positive", [BS, D], F32, kind="ExternalInput")
    n_h = nc.dram_tensor("negative", [BS, D], F32, kind="ExternalInput")
    o_h = nc.dram_tensor("out", [P, 1], F32, kind="ExternalOutput")

    # sample index s = (t*P + p)*SPT + j  ->  per-partition contiguous 8 KiB reads
    a_v = a_h.ap().rearrange("(t p j) d -> t p j d", t=NT, p=P, j=SPT)
    p_v = p_h.ap().rearrange("(t p j) d -> t p j d", t=NT, p=P, j=SPT)
    n_v = n_h.ap().rearrange("(t p j) d -> t p j d", t=NT, p=P, j=SPT)

    with tile.TileContext(nc) as tc:
        with (
            tc.tile_pool(name="inp", bufs=3) as in_pool,
            tc.tile_pool(name="w", bufs=2) as w_pool,
            tc.tile_pool(name="scr", bufs=4) as scr_pool,
            tc.tile_pool(name="acc", bufs=1) as acc_pool,
            tc.tile_pool(name="epi", bufs=1) as epi_pool,
        ):
            s11 = acc_pool.tile([P, NCOLS], F32, tag="s11")
            s22 = acc_pool.tile([P, NCOLS], F32, tag="s22")
            s12 = acc_pool.tile([P, NCOLS], F32, tag="s12")

            for t in range(NT):
                at = in_pool.tile([P, SPT, D], F32, tag="a")
                nc.sync.dma_start(at[:], a_v[t])
                pt = in_pool.tile([P, SPT, D], F32, tag="p")
                nc.sync.dma_start(pt[:], p_v[t])
                ntl = in_pool.tile([P, SPT, D], F32, tag="n")
                nc.sync.dma_start(ntl[:], n_v[t])

                af = at[:].rearrange("p j d -> p (j d)")
                pf = pt[:].rearrange("p j d -> p (j d)")
                nf = ntl[:].rearrange("p j d -> p (j d)")

                w1 = w_pool.tile([P, G], BF16, tag="w1")
                nc.vector.tensor_sub(w1[:], af, pf)
                w2 = w_pool.tile([P, G], BF16, tag="w2")
                nc.vector.tensor_sub(w2[:], af, nf)

                for j in range(SPT):
                    col = t * SPT + j
                    x1 = w1[:, j * D : (j + 1) * D]
                    x2 = w2[:, j * D : (j + 1) * D]
                    if j in DVE_SQUARE_SLICES:
                        sc1 = scr_pool.tile([P, D], BF16, tag="dsq")
                        nc.vector.scalar_tensor_tensor(
                            sc1[:], x1, 1.0, x1, Alu.mult, Alu.mult,
                            accum_out=s11[:, col : col + 1],
                        )
                        sc2 = scr_pool.tile([P, D], BF16, tag="dsq")
                        nc.vector.scalar_tensor_tensor(
                            sc2[:], x2, 1.0, x2, Alu.mult, Alu.mult,
                            accum_out=s22[:, col : col + 1],
                        )
                    else:
                        sc1 = scr_pool.tile([P, D], BF16, tag="asq")
                        nc.scalar.activation(
                            sc1[:], x1, Act.Square, accum_out=s11[:, col : col + 1]
                        )
                        sc2 = scr_pool.tile([P, D], BF16, tag="asq")
                        nc.scalar.activation(
                            sc2[:], x2, Act.Square, accum_out=s22[:, col : col + 1]
                        )
                    sc3 = scr_pool.tile([P, D], BF16, tag="dtr")
                    nc.vector.scalar_tensor_tensor(
                        sc3[:], x1, 1.0, x2, Alu.mult, Alu.mult,
                        accum_out=s12[:, col : col + 1],
                    )

            # ---- epilogue on [P, NCOLS] ----
            def etile(tag):
                return epi_pool.tile([P, NCOLS], F32, tag=tag, name=tag)

            d_ap = etile("d_ap")
            nc.scalar.activation(d_ap[:], s11[:], Act.Sqrt)
            d_an = etile("d_an")
            nc.scalar.activation(d_an[:], s22[:], Act.Sqrt)

            tmp = etile("tmp")
            nc.vector.tensor_add(tmp[:], s11[:], s22[:])
            dpn2 = etile("dpn2")
            nc.vector.scalar_tensor_tensor(
                dpn2[:], s12[:], -2.0, tmp[:], Alu.mult, Alu.add
            )
            d_pn = etile("d_pn")
            nc.scalar.activation(d_pn[:], dpn2[:], Act.Sqrt)

            e1 = etile("e1")
            nc.scalar.activation(e1[:], d_ap[:], Act.Exp, scale=4.0)
            bias4 = epi_pool.tile([P, 1], F32, tag="bias4", name="bias4")
            nc.gpsimd.memset(bias4[:], 4.0)
            e2 = etile("e2")
            nc.scalar.activation(e2[:], d_an[:], Act.Exp, bias=bias4[:], scale=-4.0)

            e1p = etile("e1p")
            nc.vector.tensor_scalar_add(e1p[:], e1[:], EPS)
            r1 = etile("r1")
            nc.vector.reciprocal(r1[:], e1p[:])
            e2p = etile("e2p")
            nc.vector.tensor_scalar_add(e2p[:], e2[:], EPS)
            r2 = etile("r2")
            nc.vector.reciprocal(r2[:], e2p[:])

            # loss - 2 = d_ap - 0.5*d_an - 0.5*d_pn + 2*r1 + 2*r2
            t1 = etile("t1")
            nc.vector.scalar_tensor_tensor(t1[:], d_an[:], -0.5, d_ap[:], Alu.mult, Alu.add)
            t2 = etile("t2")
            nc.vector.scalar_tensor_tensor(t2[:], d_pn[:], -0.5, t1[:], Alu.mult, Alu.add)
            t3 = etile("t3")
            nc.vector.scalar_tensor_tensor(t3[:], r1[:], 2.0, t2[:], Alu.mult, Alu.add)
            t4 = etile("t4")
            nc.vector.scalar_tensor_tensor(t4[:], r2[:], 2.0, t3[:], Alu.mult, Alu.add)

            row = epi_pool.tile([P, 1], F32, tag="row")
            nc.vector.tensor_reduce(row[:], t4[:], axis=AX.X, op=Alu.add)
            nc.sync.dma_start(o_h.ap(), row[:])

    nc.compile()
    return nc


def _get_nc():
    if "nc" not in _CACHE:
        _CACHE["nc"] = _build()
    return _CACHE["nc"]


def kernel(anchor, positive, negative, _trace=False):
    nc = _get_nc()
    in_maps = []
    for i in range(NCORES):
        sl = slice(i * BS, (i + 1) * BS)
        in_maps.append(
            {
                "anchor": np.ascontiguousarray(anchor[sl], dtype=np.float32),
                "positive": np.ascontiguousarray(positive[sl], dtype=np.float32),
                "negative": np.ascontiguousarray(negative[sl], dtype=np.float32),
            }
        )
    res = bass_utils.run_bass_kernel_spmd(
        nc, in_maps, core_ids=list(range(NCORES)), trace=_trace
    )
    _CACHE["last_result"] = res
    total = np.float64(0.0)
    for r in res.results:
        total += np.asarray(r["out"], dtype=np.float64).sum()
    mean = total / B + 2.0
    return np.array(mean, dtype=np.float32)
